# revision 3
# baseline (speedup 1.0000x reference)
"""Trainium2 Bass kernel: grouped MoE expert MLP (nn_ExpertGroup).

Strategy: expert parallelism across 8 NeuronCores. Tokens are sorted by
expert; core e runs expert e's two GEMMs:
    h = relu(x_e @ w_up[e].T) ** 2      (bf16, like the CUDA reference)
    y = h @ w_down[e].T
The host does the (free) token scatter/gather, the bf16 casts, and the
weight transposes/swizzles so every device-side DMA is a single
contiguous run per partition (128 descriptors; small-segment DMAs halve
bandwidth and cost ~5ns/descriptor of engine issue time).

Timing model (measured): ~6.4us fixed runtime prologue, then DMA issue
on two parallel HWDGE rings (Sync + Scalar queues), PE warmup matmuls
until the first GEMM1 operands land (~11.5us), 109.2us dense bf16 PE
stream (the roofline: 2 x 1024x1024x2048 MACs @ 16384 MAC/cycle,
2.4 GHz), then a short drain + teardown tail.

Device layout (per core, cap = padded local token count, default 1024):
    xT  swizzled [128, n_c1, 8, FD1] bf16  (c-chunk, d-tile, tokens)
    wuT swizzled [128, 16, 8, 128]   bf16  (j-tile, d-tile, j-cols)
    wdT swizzled [128, 16, 1024]     bf16  (j-tile, output cols)
    GEMM1: psum[j,t] = sum_d wuT[:,j,d].T @ xT[:,c,d]   (h in [H, T] layout)
    ACT:   relu fp32 psum -> bf16 hr; DVE: square -> hsq [128, 16, cap]
    GEMM2: psum[t,i] = sum_j hsq[j,t].T @ wdT[:,j,i]  (y in [T, D] layout)
    DVE:   cast fp32 psum -> bf16 y -> DMA out

Precision: bf16 everywhere (matches the reference's bf16 pipeline,
rel err ~5e-3). fp8 DoubleRow (2x PE) was evaluated and rejected: e4m3
quantization is ~2.7% rms per operand; uncompensated error is ~5.8e-2
(gate 2e-2) and full error-compensation costs 1.5x bf16 PE time.

Built on bacc.Bacc (not raw Bass): Bacc.compile() legalizes semaphore
waits to the TRN2 limit of one wait per instruction.
"""

import numpy as np
import ml_dtypes

import concourse.bass as bass
import concourse.mybir as mybir
import concourse.tile as tile
from concourse import bacc
from concourse.bass_utils import run_bass_kernel_spmd

T, D, H, E = 8192, 1024, 2048, 8
P = 128
N_CORES = 8
FD1 = 512  # GEMM1 moving free dim (token chunk)
FD2 = 512  # GEMM2 moving free dim (one PSUM bank of fp32)
WARM_N = 26
n_d_host = D // P


def _ensure_axon_ntff_hook():
    """The container's `antenv` stub lacks `axon_hooks`; if BASS_TRACE=1 is
    set, run_bass_kernel_spmd would crash importing it. Recreate the tiny
    registry and register the ctypes NTFF hook so tracing works (and never
    let this best-effort setup break the kernel)."""
    try:
        import antenv.axon_hooks  # noqa: F401
        return
    except ImportError:
        pass
    try:
        import sys
        import types

        import antenv
        from trn_agent_boot.trn_boot import _ntff_profile_via_ctypes

        mod = types.ModuleType("antenv.axon_hooks")
        mod._hook = _ntff_profile_via_ctypes("/opt/axon/libaxon_pjrt.so")
        mod.set_axon_ntff_profile_hook = lambda h: setattr(mod, "_hook", h)
        mod.get_axon_ntff_profile_hook = lambda: mod._hook
        sys.modules["antenv.axon_hooks"] = mod
        antenv.axon_hooks = mod
    except Exception:
        pass


_ensure_axon_ntff_hook()

_PROGRAM_CACHE: dict[int, "bass.Bass"] = {}
LAST_RESULT = None  # BassKernelResults of the most recent run (for harness use)


def _build_program(cap: int) -> "bass.Bass":
    assert cap % FD2 == 0 and cap % FD1 == 0
    n_d = D // P       # 8  contraction tiles of GEMM1
    n_j = H // P       # 16 H partition tiles
    n_c1 = cap // FD1  # GEMM1 token chunks (moving operand)
    n_t = cap // P     # token partition tiles (GEMM2 output)
    n_ic = D // FD2    # GEMM2 output column chunks
    bf16 = mybir.dt.bfloat16
    f32 = mybir.dt.float32

    nc = bacc.Bacc("TRN2", debug=False, num_devices=N_CORES)
    xT = nc.dram_tensor("xT", [P, n_c1, n_d, FD1], bf16, kind="ExternalInput")
    wuT = nc.dram_tensor("wuT", [P, n_j, n_d, P], bf16, kind="ExternalInput")
    wdT = nc.dram_tensor("wdT", [P, n_j, D], bf16, kind="ExternalInput")
    y = nc.dram_tensor("y", [cap, D], bf16, kind="ExternalOutput")

    y3 = y[:].rearrange("(po pi) f -> pi po f", pi=P)  # [128, n_t, 1024]

    with tile.TileContext(nc) as tc:
        with (
            tc.tile_pool(name="big", bufs=1) as big,
            tc.tile_pool(name="outp", bufs=4) as outp,
            tc.tile_pool(name="actp", bufs=4) as actp,
            tc.tile_pool(name="psum", bufs=6, space="PSUM") as psum,
            tc.tile_pool(name="warmp", bufs=1, space="PSUM") as warmp,
        ):
            xT_sb = big.tile([P, n_c1, n_d, FD1], bf16)
            wuT_sb = big.tile([P, n_j, n_d, P], bf16)
            wdT_sb = big.tile([P, n_j, D], bf16)
            hsq_sb = big.tile([P, n_j, cap], bf16)

            # PE warm-up: dummy matmuls with no DMA dependencies run while
            # the input DMAs stream in, keeping the PE busy through the HAM
            # activity window so the real matmul stream starts at the full
            # 2.4 GHz clock instead of the cold 1.2 GHz.
            warm = big.tile([P, 256], bf16)
            nc.vector.memset(warm[:], 0.0)
            wps = warmp.tile([P, 256], f32, tag="warm")
            for _ in range(WARM_N):
                nc.tensor.matmul(wps, warm[:, 0:P], warm[:], start=True, stop=True)

            # Input DMAs on two parallel HWDGE rings. Sync carries w_up in
            # consumption-ordered pieces; Scalar carries x then w_down.
            # Every transfer is one contiguous run per partition.
            nc.sync.dma_start(out=wuT_sb[:, 0], in_=wuT[:, 0])
            nc.scalar.dma_start(out=xT_sb[:, 0], in_=xT[:, 0])
            nc.sync.dma_start(out=wuT_sb[:, 1], in_=wuT[:, 1])
            nc.sync.dma_start(out=wuT_sb[:, 2:4], in_=wuT[:, 2:4])
            for c in range(1, n_c1):
                nc.scalar.dma_start(out=xT_sb[:, c], in_=xT[:, c])
            nc.sync.dma_start(out=wuT_sb[:, 4:8], in_=wuT[:, 4:8])
            nc.sync.dma_start(out=wuT_sb[:, 8:12], in_=wuT[:, 8:12])
            nc.scalar.dma_start(out=wdT_sb[:], in_=wdT[:])
            nc.sync.dma_start(out=wuT_sb[:, 12:16], in_=wuT[:, 12:16])

            # GEMM1 + relu^2: hsq[j, t] in SBUF, token-chunk outer. ACT
            # (scalar engine) does the relu+downcast, DVE the square, so
            # neither engine is near saturation.
            for c in range(n_c1):
                for j in range(n_j):
                    ps = psum.tile([P, FD1], f32, tag="ps")
                    for d in range(n_d):
                        nc.tensor.matmul(
                            ps,
                            wuT_sb[:, j, d],
                            xT_sb[:, c, d],
                            start=(d == 0),
                            stop=(d == n_d - 1),
                        )
                    hr = actp.tile([P, FD1], bf16, tag="hr")
                    nc.scalar.activation(
                        out=hr, in_=ps,
                        func=mybir.ActivationFunctionType.Relu,
                    )
                    nc.vector.tensor_mul(
                        out=hsq_sb[:, j, c * FD1:(c + 1) * FD1], in0=hr, in1=hr
                    )

            # GEMM2: y[t, i] = sum_j hsq[j, t].T @ wdT[j, i]. The final
            # (t, ic) tile is split in half so the last output DMA is small
            # and issues earlier, shortening the post-PE tail.
            for t in range(n_t):
                for ic in range(n_ic):
                    last = (t == n_t - 1) and (ic == n_ic - 1)
                    pieces = (
                        [(ic * FD2, FD2)] if not last
                        else [(ic * FD2, FD2 // 2),
                              (ic * FD2 + FD2 // 2, FD2 // 2)]
                    )
                    for (i0, fw) in pieces:
                        ps = psum.tile([P, FD2], f32, tag="ps")
                        for j in range(n_j):
                            nc.tensor.matmul(
                                ps[:, 0:fw],
                                hsq_sb[:, j, t * P:(t + 1) * P],
                                wdT_sb[:, j, i0:i0 + fw],
                                start=(j == 0),
                                stop=(j == n_j - 1),
                            )
                        yt = outp.tile([P, FD2], bf16, tag="yt")
                        nc.vector.tensor_copy(out=yt[:, 0:fw], in_=ps[:, 0:fw])
                        nc.sync.dma_start(
                            out=y3[:, t, i0:i0 + fw], in_=yt[:, 0:fw]
                        )

    nc.compile()
    return nc


def _get_program(cap: int) -> "bass.Bass":
    nc = _PROGRAM_CACHE.get(cap)
    if nc is None:
        nc = _build_program(cap)
        _PROGRAM_CACHE[cap] = nc
    return nc


CAP = 1024  # tokens per core per round (the uniform T/E split = one round)


def _swizzle_x(xs):
    """[cap, D] bf16 -> [128, n_c1, 8, FD1] with xT[p, c, d, t] =
    xs[c*FD1 + t, d*128 + p]."""
    cap = xs.shape[0]
    a = xs.reshape(cap // FD1, FD1, n_d_host, P)  # (c, t, d, p)
    return np.ascontiguousarray(a.transpose(3, 0, 2, 1))


def _swizzle_wu(wu8):
    """[H, D] bf16 -> [128, 16, 8, 128] with wuT[p, j, d, t] =
    wu8[j*128 + t, d*128 + p]."""
    a = wu8.reshape(H // P, P, n_d_host, P)  # (j, t, d, p)
    return np.ascontiguousarray(a.transpose(3, 0, 2, 1))


def _swizzle_wd(wd8):
    """[D, H] bf16 -> [128, 16, 1024] with wdT[p, j, i] =
    wd8[i, j*128 + p]."""
    a = wd8.T.reshape(H // P, P, D)  # (j, p, i)
    return np.ascontiguousarray(a.transpose(1, 0, 2))


def kernel(x, num_tokens_per_expert, w_up, w_down, _trace=False):
    global LAST_RESULT
    bf = ml_dtypes.bfloat16
    x = np.asarray(x)
    counts = np.asarray(num_tokens_per_expert).astype(np.int64)
    w_up = np.asarray(w_up)
    w_down = np.asarray(w_down)
    n_tok = x.shape[0]
    assert counts.shape == (E,) and int(counts.sum()) == n_tok
    offsets = np.zeros(E, dtype=np.int64)
    offsets[1:] = np.cumsum(counts)[:-1]

    nc = _get_program(CAP)

    # Work list: split each expert's contiguous token segment into slots of
    # <= CAP tokens; process 8 slots per SPMD round. The uniform T/E = 1024
    # split is exactly one round of 8 slots.
    slots = []
    for e in range(E):
        cnt, off = int(counts[e]), int(offsets[e])
        for s in range(0, cnt, CAP):
            slots.append((e, off + s, min(CAP, cnt - s)))

    wuT_cache = {}
    wdT_cache = {}

    def expert_weights(e):
        if e not in wuT_cache:
            wuT_cache[e] = _swizzle_wu(w_up[e].astype(bf))
            wdT_cache[e] = _swizzle_wd(w_down[e].astype(bf))
        return wuT_cache[e], wdT_cache[e]

    out = np.zeros((n_tok, D), dtype=x.dtype)
    zero_map = None
    for r0 in range(0, len(slots), N_CORES):
        round_slots = slots[r0:r0 + N_CORES]
        in_maps = []
        for e, off, cnt in round_slots:
            xs = np.zeros((CAP, D), dtype=bf)
            xs[:cnt] = x[off:off + cnt].astype(bf)
            wuT, wdT = expert_weights(e)
            in_maps.append({
                "xT": _swizzle_x(xs), "wuT": wuT, "wdT": wdT,
            })
        while len(in_maps) < N_CORES:  # idle cores in the last round
            if zero_map is None:
                zero_map = {
                    "xT": np.zeros((P, CAP // FD1, n_d_host, FD1), dtype=bf),
                    "wuT": np.zeros((P, H // P, n_d_host, P), dtype=bf),
                    "wdT": np.zeros((P, H // P, D), dtype=bf),
                }
            in_maps.append(zero_map)

        res = run_bass_kernel_spmd(
            nc, in_maps, core_ids=list(range(N_CORES)), trace=_trace
        )
        LAST_RESULT = res
        for i, (e, off, cnt) in enumerate(round_slots):
            out[off:off + cnt] = res.results[i]["y"][:cnt].astype(x.dtype)
    return out


# revision 4
# speedup vs baseline: 1.0470x; 1.0470x over previous
"""Trainium2 Bass kernel: grouped MoE expert MLP (nn_ExpertGroup).

Strategy: expert parallelism across 8 NeuronCores. Tokens are sorted by
expert; core e runs expert e's two GEMMs:
    h = relu(x_e @ w_up[e].T) ** 2      (bf16, like the CUDA reference)
    y = h @ w_down[e].T
The host does the (free) token scatter/gather, the bf16 casts, and the
weight transposes/swizzles so every device-side DMA is a single
contiguous run per partition (128 descriptors; small-segment DMAs halve
bandwidth and cost ~5ns/descriptor of engine issue time).

Timing model (measured): ~6.4us fixed runtime prologue, then DMA issue
on two parallel HWDGE rings (Sync + Scalar queues), PE warmup matmuls
until the first GEMM1 operands land (~11.5us), 109.2us dense bf16 PE
stream (the roofline: 2 x 1024x1024x2048 MACs @ 16384 MAC/cycle,
2.4 GHz), then a short drain + teardown tail.

Device layout (per core, cap = padded local token count, default 1024):
    xT  swizzled [128, n_c1, 8, FD1] bf16  (c-chunk, d-tile, tokens)
    wuT swizzled [128, 16, 8, 128]   bf16  (j-tile, d-tile, j-cols)
    wdT swizzled [128, 16, 1024]     bf16  (j-tile, output cols)
    GEMM1: psum[j,t] = sum_d wuT[:,j,d].T @ xT[:,c,d]   (h in [H, T] layout)
    ACT:   relu fp32 psum -> bf16 hr; DVE: square -> hsq [128, 16, cap]
    GEMM2: psum[t,i] = sum_j hsq[j,t].T @ wdT[:,j,i]  (y in [T, D] layout)
    DVE:   cast fp32 psum -> bf16 y -> DMA out

Precision: bf16 everywhere (matches the reference's bf16 pipeline,
rel err ~5e-3). fp8 DoubleRow (2x PE) was evaluated and rejected: e4m3
quantization is ~2.7% rms per operand; uncompensated error is ~5.8e-2
(gate 2e-2) and full error-compensation costs 1.5x bf16 PE time.

Built on bacc.Bacc (not raw Bass): Bacc.compile() legalizes semaphore
waits to the TRN2 limit of one wait per instruction.
"""

import numpy as np
import ml_dtypes

import concourse.bass as bass
import concourse.mybir as mybir
import concourse.tile as tile
from concourse import bacc
from concourse.bass_utils import run_bass_kernel_spmd

T, D, H, E = 8192, 1024, 2048, 8
P = 128
N_CORES = 8
FD1 = 512  # GEMM1 moving free dim (token chunk)
FD2 = 512  # GEMM2 moving free dim (one PSUM bank of fp32)
WARM_N = 26
n_d_host = D // P


def _ensure_axon_ntff_hook():
    """The container's `antenv` stub lacks `axon_hooks`; if BASS_TRACE=1 is
    set, run_bass_kernel_spmd would crash importing it. Recreate the tiny
    registry and register the ctypes NTFF hook so tracing works (and never
    let this best-effort setup break the kernel)."""
    try:
        import antenv.axon_hooks  # noqa: F401
        return
    except ImportError:
        pass
    try:
        import sys
        import types

        import antenv
        from trn_agent_boot.trn_boot import _ntff_profile_via_ctypes

        mod = types.ModuleType("antenv.axon_hooks")
        mod._hook = _ntff_profile_via_ctypes("/opt/axon/libaxon_pjrt.so")
        mod.set_axon_ntff_profile_hook = lambda h: setattr(mod, "_hook", h)
        mod.get_axon_ntff_profile_hook = lambda: mod._hook
        sys.modules["antenv.axon_hooks"] = mod
        antenv.axon_hooks = mod
    except Exception:
        pass


_ensure_axon_ntff_hook()

_PROGRAM_CACHE: dict[int, "bass.Bass"] = {}
LAST_RESULT = None  # BassKernelResults of the most recent run (for harness use)


def _build_program(cap: int) -> "bass.Bass":
    assert cap % FD2 == 0 and cap % FD1 == 0
    n_d = D // P       # 8  contraction tiles of GEMM1
    n_j = H // P       # 16 H partition tiles
    n_c1 = cap // FD1  # GEMM1 token chunks (moving operand)
    n_t = cap // P     # token partition tiles (GEMM2 output)
    n_ic = D // FD2    # GEMM2 output column chunks
    bf16 = mybir.dt.bfloat16
    f32 = mybir.dt.float32

    nc = bacc.Bacc("TRN2", debug=False, num_devices=N_CORES)
    xT = nc.dram_tensor("xT", [P, n_c1, n_d, FD1], bf16, kind="ExternalInput")
    wuT = nc.dram_tensor("wuT", [P, n_j, n_d, P], bf16, kind="ExternalInput")
    wdT = nc.dram_tensor("wdT", [P, n_j, D], bf16, kind="ExternalInput")
    y = nc.dram_tensor("y", [cap, D], bf16, kind="ExternalOutput")

    y3 = y[:].rearrange("(po pi) f -> pi po f", pi=P)  # [128, n_t, 1024]

    with tile.TileContext(nc) as tc:
        with (
            tc.tile_pool(name="big", bufs=1) as big,
            tc.tile_pool(name="outp", bufs=4) as outp,
            tc.tile_pool(name="actp", bufs=4) as actp,
            tc.tile_pool(name="psum", bufs=6, space="PSUM") as psum,
            tc.tile_pool(name="warmp", bufs=1, space="PSUM") as warmp,
        ):
            xT_sb = big.tile([P, n_c1, n_d, FD1], bf16)
            wuT_sb = big.tile([P, n_j, n_d, P], bf16)
            wdT_sb = big.tile([P, n_j, D], bf16)
            hsq_sb = big.tile([P, n_j, cap], bf16)

            # PE warm-up: dummy matmuls with no DMA dependencies run while
            # the input DMAs stream in, keeping the PE busy through the HAM
            # activity window so the real matmul stream starts at the full
            # 2.4 GHz clock instead of the cold 1.2 GHz.
            warm = big.tile([P, 256], bf16)
            nc.vector.memset(warm[:], 0.0)
            wps = warmp.tile([P, 256], f32, tag="warm")
            for _ in range(WARM_N):
                nc.tensor.matmul(wps, warm[:, 0:P], warm[:], start=True, stop=True)

            # Input DMAs on two parallel HWDGE rings. Sync carries w_up in
            # consumption-ordered pieces; Scalar carries x then w_down.
            # Every transfer is one contiguous run per partition.
            nc.sync.dma_start(out=wuT_sb[:, 0], in_=wuT[:, 0])
            nc.scalar.dma_start(out=xT_sb[:, 0], in_=xT[:, 0])
            nc.sync.dma_start(out=wuT_sb[:, 1], in_=wuT[:, 1])
            nc.sync.dma_start(out=wuT_sb[:, 2:4], in_=wuT[:, 2:4])
            for c in range(1, n_c1):
                nc.scalar.dma_start(out=xT_sb[:, c], in_=xT[:, c])
            nc.sync.dma_start(out=wuT_sb[:, 4:8], in_=wuT[:, 4:8])
            nc.sync.dma_start(out=wuT_sb[:, 8:12], in_=wuT[:, 8:12])
            nc.sync.dma_start(out=wuT_sb[:, 12:16], in_=wuT[:, 12:16])
            # w_down is not needed until GEMM2 (~57us in); issuing it last on
            # the sync ring keeps its packets from stealing SDMA bandwidth
            # from the w_up stream that gates GEMM1.
            nc.sync.dma_start(out=wdT_sb[:], in_=wdT[:])

            # GEMM1 + relu^2: hsq[j, t] in SBUF, token-chunk outer. ACT
            # (scalar engine) does the relu+downcast, DVE the square, so
            # neither engine is near saturation.
            for c in range(n_c1):
                for j in range(n_j):
                    ps = psum.tile([P, FD1], f32, tag="ps")
                    for d in range(n_d):
                        nc.tensor.matmul(
                            ps,
                            wuT_sb[:, j, d],
                            xT_sb[:, c, d],
                            start=(d == 0),
                            stop=(d == n_d - 1),
                        )
                    hr = actp.tile([P, FD1], bf16, tag="hr")
                    nc.vector.tensor_relu(out=hr, in_=ps)
                    nc.vector.tensor_mul(
                        out=hsq_sb[:, j, c * FD1:(c + 1) * FD1], in0=hr, in1=hr
                    )

            # GEMM2: y[t, i] = sum_j hsq[j, t].T @ wdT[j, i]. The final
            # (t, ic) tile is split in half so the last output DMA is small
            # and issues earlier, shortening the post-PE tail.
            for t in range(n_t):
                for ic in range(n_ic):
                    ps = psum.tile([P, FD2], f32, tag="ps")
                    for j in range(n_j):
                        nc.tensor.matmul(
                            ps,
                            hsq_sb[:, j, t * P:(t + 1) * P],
                            wdT_sb[:, j, ic * FD2:(ic + 1) * FD2],
                            start=(j == 0),
                            stop=(j == n_j - 1),
                        )
                    yt = outp.tile([P, FD2], bf16, tag="yt")
                    last = (t == n_t - 1) and (ic == n_ic - 1)
                    if not last:
                        nc.vector.tensor_copy(out=yt, in_=ps)
                        nc.sync.dma_start(out=y3[:, t, ic * FD2:(ic + 1) * FD2], in_=yt)
                    else:
                        # split the final drain so the last (small) output
                        # DMA issues as early as possible
                        h = FD2 // 2
                        nc.vector.tensor_copy(out=yt[:, 0:h], in_=ps[:, 0:h])
                        nc.sync.dma_start(
                            out=y3[:, t, ic * FD2:ic * FD2 + h], in_=yt[:, 0:h])
                        nc.vector.tensor_copy(out=yt[:, h:FD2], in_=ps[:, h:FD2])
                        nc.sync.dma_start(
                            out=y3[:, t, ic * FD2 + h:(ic + 1) * FD2], in_=yt[:, h:FD2])

    nc.compile()
    return nc


def _get_program(cap: int) -> "bass.Bass":
    nc = _PROGRAM_CACHE.get(cap)
    if nc is None:
        nc = _build_program(cap)
        _PROGRAM_CACHE[cap] = nc
    return nc


CAP = 1024  # tokens per core per round (the uniform T/E split = one round)


def _swizzle_x(xs):
    """[cap, D] bf16 -> [128, n_c1, 8, FD1] with xT[p, c, d, t] =
    xs[c*FD1 + t, d*128 + p]."""
    cap = xs.shape[0]
    a = xs.reshape(cap // FD1, FD1, n_d_host, P)  # (c, t, d, p)
    return np.ascontiguousarray(a.transpose(3, 0, 2, 1))


def _swizzle_wu(wu8):
    """[H, D] bf16 -> [128, 16, 8, 128] with wuT[p, j, d, t] =
    wu8[j*128 + t, d*128 + p]."""
    a = wu8.reshape(H // P, P, n_d_host, P)  # (j, t, d, p)
    return np.ascontiguousarray(a.transpose(3, 0, 2, 1))


def _swizzle_wd(wd8):
    """[D, H] bf16 -> [128, 16, 1024] with wdT[p, j, i] =
    wd8[i, j*128 + p]."""
    a = wd8.T.reshape(H // P, P, D)  # (j, p, i)
    return np.ascontiguousarray(a.transpose(1, 0, 2))


def kernel(x, num_tokens_per_expert, w_up, w_down, _trace=False):
    global LAST_RESULT
    bf = ml_dtypes.bfloat16
    x = np.asarray(x)
    counts = np.asarray(num_tokens_per_expert).astype(np.int64)
    w_up = np.asarray(w_up)
    w_down = np.asarray(w_down)
    n_tok = x.shape[0]
    assert counts.shape == (E,) and int(counts.sum()) == n_tok
    offsets = np.zeros(E, dtype=np.int64)
    offsets[1:] = np.cumsum(counts)[:-1]

    nc = _get_program(CAP)

    # Work list: split each expert's contiguous token segment into slots of
    # <= CAP tokens; process 8 slots per SPMD round. The uniform T/E = 1024
    # split is exactly one round of 8 slots.
    slots = []
    for e in range(E):
        cnt, off = int(counts[e]), int(offsets[e])
        for s in range(0, cnt, CAP):
            slots.append((e, off + s, min(CAP, cnt - s)))

    wuT_cache = {}
    wdT_cache = {}

    def expert_weights(e):
        if e not in wuT_cache:
            wuT_cache[e] = _swizzle_wu(w_up[e].astype(bf))
            wdT_cache[e] = _swizzle_wd(w_down[e].astype(bf))
        return wuT_cache[e], wdT_cache[e]

    out = np.zeros((n_tok, D), dtype=x.dtype)
    zero_map = None
    for r0 in range(0, len(slots), N_CORES):
        round_slots = slots[r0:r0 + N_CORES]
        in_maps = []
        for e, off, cnt in round_slots:
            xs = np.zeros((CAP, D), dtype=bf)
            xs[:cnt] = x[off:off + cnt].astype(bf)
            wuT, wdT = expert_weights(e)
            in_maps.append({
                "xT": _swizzle_x(xs), "wuT": wuT, "wdT": wdT,
            })
        while len(in_maps) < N_CORES:  # idle cores in the last round
            if zero_map is None:
                zero_map = {
                    "xT": np.zeros((P, CAP // FD1, n_d_host, FD1), dtype=bf),
                    "wuT": np.zeros((P, H // P, n_d_host, P), dtype=bf),
                    "wdT": np.zeros((P, H // P, D), dtype=bf),
                }
            in_maps.append(zero_map)

        res = run_bass_kernel_spmd(
            nc, in_maps, core_ids=list(range(N_CORES)), trace=_trace
        )
        LAST_RESULT = res
        for i, (e, off, cnt) in enumerate(round_slots):
            out[off:off + cnt] = res.results[i]["y"][:cnt].astype(x.dtype)
    return out


# revision 5
# speedup vs baseline: 1.0804x; 1.0319x over previous
"""Trainium2 Bass kernel: grouped MoE expert MLP (nn_ExpertGroup).

Strategy: expert parallelism across 8 NeuronCores. Tokens are sorted by
expert; core e runs expert e's two GEMMs:
    h = relu(x_e @ w_up[e].T) ** 2      (bf16, like the CUDA reference)
    y = h @ w_down[e].T
The host does the (free) token scatter/gather, the bf16 casts, and the
weight transposes/swizzles so every device-side DMA is a single
contiguous run per partition (128 descriptors; small-segment DMAs halve
bandwidth and cost ~5ns/descriptor of engine issue time).

Timing model (measured): ~6.4us fixed runtime prologue, then DMA issue
on two parallel HWDGE rings (Sync + Scalar queues), PE warmup matmuls
until the first GEMM1 operands land (~11.5us), 109.2us dense bf16 PE
stream (the roofline: 2 x 1024x1024x2048 MACs @ 16384 MAC/cycle,
2.4 GHz), then a short drain + teardown tail.

Device layout (per core, cap = padded local token count, default 1024):
    xT  swizzled [128, n_c1, 8, FD1] bf16  (c-chunk, d-tile, tokens)
    wuT swizzled [128, 16, 8, 128]   bf16  (j-tile, d-tile, j-cols)
    wdT swizzled [128, 16, 1024]     bf16  (j-tile, output cols)
    GEMM1: psum[j,t] = sum_d wuT[:,j,d].T @ xT[:,c,d]   (h in [H, T] layout)
    ACT:   relu fp32 psum -> bf16 hr; DVE: square -> hsq [128, 16, cap]
    GEMM2: psum[t,i] = sum_j hsq[j,t].T @ wdT[:,j,i]  (y in [T, D] layout)
    DVE:   cast fp32 psum -> bf16 y -> DMA out

Precision: bf16 everywhere (matches the reference's bf16 pipeline,
rel err ~5e-3). fp8 DoubleRow (2x PE) was evaluated and rejected: e4m3
quantization is ~2.7% rms per operand; uncompensated error is ~5.8e-2
(gate 2e-2) and full error-compensation costs 1.5x bf16 PE time.

Built on bacc.Bacc (not raw Bass): Bacc.compile() legalizes semaphore
waits to the TRN2 limit of one wait per instruction.
"""

import numpy as np
import ml_dtypes

import concourse.bass as bass
import concourse.mybir as mybir
import concourse.tile as tile
from concourse import bacc
from concourse.bass_utils import run_bass_kernel_spmd

T, D, H, E = 8192, 1024, 2048, 8
P = 128
N_CORES = 8
FD1 = 512  # GEMM1 moving free dim (token chunk)
FD2 = 512  # GEMM2 moving free dim (one PSUM bank of fp32)
WARM_N = 22
n_d_host = D // P


def _ensure_axon_ntff_hook():
    """The container's `antenv` stub lacks `axon_hooks`; if BASS_TRACE=1 is
    set, run_bass_kernel_spmd would crash importing it. Recreate the tiny
    registry and register the ctypes NTFF hook so tracing works (and never
    let this best-effort setup break the kernel)."""
    try:
        import antenv.axon_hooks  # noqa: F401
        return
    except ImportError:
        pass
    try:
        import sys
        import types

        import antenv
        from trn_agent_boot.trn_boot import _ntff_profile_via_ctypes

        mod = types.ModuleType("antenv.axon_hooks")
        mod._hook = _ntff_profile_via_ctypes("/opt/axon/libaxon_pjrt.so")
        mod.set_axon_ntff_profile_hook = lambda h: setattr(mod, "_hook", h)
        mod.get_axon_ntff_profile_hook = lambda: mod._hook
        sys.modules["antenv.axon_hooks"] = mod
        antenv.axon_hooks = mod
    except Exception:
        pass


_ensure_axon_ntff_hook()

_PROGRAM_CACHE: dict[int, "bass.Bass"] = {}
LAST_RESULT = None  # BassKernelResults of the most recent run (for harness use)


def _build_program(cap: int) -> "bass.Bass":
    assert cap % FD2 == 0 and cap % FD1 == 0
    n_d = D // P       # 8  contraction tiles of GEMM1
    n_j = H // P       # 16 H partition tiles
    n_c1 = cap // FD1  # GEMM1 token chunks (moving operand)
    n_t = cap // P     # token partition tiles (GEMM2 output)
    n_ic = D // FD2    # GEMM2 output column chunks
    bf16 = mybir.dt.bfloat16
    f32 = mybir.dt.float32

    nc = bacc.Bacc("TRN2", debug=False, num_devices=N_CORES)
    xT = nc.dram_tensor("xT", [P, n_c1, n_d, FD1], bf16, kind="ExternalInput")
    wuT = nc.dram_tensor("wuT", [P, n_j, n_d, P], bf16, kind="ExternalInput")
    wdT = nc.dram_tensor("wdT", [P, n_j, D], bf16, kind="ExternalInput")
    y = nc.dram_tensor("y", [cap, D], bf16, kind="ExternalOutput")

    y3 = y[:].rearrange("(po pi) f -> pi po f", pi=P)  # [128, n_t, 1024]

    with tile.TileContext(nc) as tc:
        with (
            tc.tile_pool(name="big", bufs=1) as big,
            tc.tile_pool(name="outp", bufs=4) as outp,
            tc.tile_pool(name="actp", bufs=4) as actp,
            tc.tile_pool(name="psum", bufs=6, space="PSUM") as psum,
            tc.tile_pool(name="warmp", bufs=1, space="PSUM") as warmp,
        ):
            xT_sb = big.tile([P, n_c1, n_d, FD1], bf16)
            wuT_sb = big.tile([P, n_j, n_d, P], bf16)
            wdT_sb = big.tile([P, n_j, D], bf16)
            hsq_sb = big.tile([P, n_j, cap], bf16)

            # PE warm-up: dummy matmuls with no DMA dependencies run while
            # the input DMAs stream in, keeping the PE busy through the HAM
            # activity window so the real matmul stream starts at the full
            # 2.4 GHz clock instead of the cold 1.2 GHz.
            warm = big.tile([P, 256], bf16)
            nc.vector.memset(warm[:], 0.0)
            wps = warmp.tile([P, 256], f32, tag="warm")
            for _ in range(WARM_N):
                nc.tensor.matmul(wps, warm[:, 0:P], warm[:], start=True, stop=True)

            # Input DMAs: ONE ring (sync queue), just-in-time FIFO order.
            # Splitting across the two HWDGE rings halves each stream's
            # bandwidth (the 16 SDMA engines round-robin between rings), so
            # a single full-rate stream ordered by consumption time wins:
            # x chunk 0 gates the first matmul; each wu piece k is consumed
            # 0.86us after piece k-1 but delivers in 0.72us; x chunk 1 and
            # w_down trail (needed at ~27us / ~70us).
            nc.sync.dma_start(out=xT_sb[:, 0], in_=xT[:, 0])
            nc.sync.dma_start(out=wuT_sb[:, 0], in_=wuT[:, 0])
            nc.sync.dma_start(out=wuT_sb[:, 1], in_=wuT[:, 1])
            nc.sync.dma_start(out=wuT_sb[:, 2:4], in_=wuT[:, 2:4])
            nc.sync.dma_start(out=wuT_sb[:, 4:8], in_=wuT[:, 4:8])
            nc.sync.dma_start(out=wuT_sb[:, 8:12], in_=wuT[:, 8:12])
            nc.sync.dma_start(out=wuT_sb[:, 12:16], in_=wuT[:, 12:16])
            for c in range(1, n_c1):
                nc.sync.dma_start(out=xT_sb[:, c], in_=xT[:, c])
            nc.sync.dma_start(out=wdT_sb[:], in_=wdT[:])

            # GEMM1 + relu^2: hsq[j, t] in SBUF, token-chunk outer. ACT
            # (scalar engine) does the relu+downcast, DVE the square, so
            # neither engine is near saturation.
            for c in range(n_c1):
                for j in range(n_j):
                    ps = psum.tile([P, FD1], f32, tag="ps")
                    for d in range(n_d):
                        nc.tensor.matmul(
                            ps,
                            wuT_sb[:, j, d],
                            xT_sb[:, c, d],
                            start=(d == 0),
                            stop=(d == n_d - 1),
                        )
                    hr = actp.tile([P, FD1], bf16, tag="hr")
                    nc.vector.tensor_relu(out=hr, in_=ps)
                    nc.vector.tensor_mul(
                        out=hsq_sb[:, j, c * FD1:(c + 1) * FD1], in0=hr, in1=hr
                    )

            # GEMM2: y[t, i] = sum_j hsq[j, t].T @ wdT[j, i]. The final
            # (t, ic) tile is split in half so the last output DMA is small
            # and issues earlier, shortening the post-PE tail.
            for t in range(n_t):
                for ic in range(n_ic):
                    ps = psum.tile([P, FD2], f32, tag="ps")
                    for j in range(n_j):
                        nc.tensor.matmul(
                            ps,
                            hsq_sb[:, j, t * P:(t + 1) * P],
                            wdT_sb[:, j, ic * FD2:(ic + 1) * FD2],
                            start=(j == 0),
                            stop=(j == n_j - 1),
                        )
                    yt = outp.tile([P, FD2], bf16, tag="yt")
                    last = (t == n_t - 1) and (ic == n_ic - 1)
                    if not last:
                        nc.vector.tensor_copy(out=yt, in_=ps)
                        nc.sync.dma_start(out=y3[:, t, ic * FD2:(ic + 1) * FD2], in_=yt)
                    else:
                        # split the final drain so the last (small) output
                        # DMA issues as early as possible
                        h = FD2 // 2
                        nc.vector.tensor_copy(out=yt[:, 0:h], in_=ps[:, 0:h])
                        nc.sync.dma_start(
                            out=y3[:, t, ic * FD2:ic * FD2 + h], in_=yt[:, 0:h])
                        nc.vector.tensor_copy(out=yt[:, h:FD2], in_=ps[:, h:FD2])
                        nc.sync.dma_start(
                            out=y3[:, t, ic * FD2 + h:(ic + 1) * FD2], in_=yt[:, h:FD2])

    nc.compile()
    return nc


def _get_program(cap: int) -> "bass.Bass":
    nc = _PROGRAM_CACHE.get(cap)
    if nc is None:
        nc = _build_program(cap)
        _PROGRAM_CACHE[cap] = nc
    return nc


CAP = 1024  # tokens per core per round (the uniform T/E split = one round)


def _swizzle_x(xs):
    """[cap, D] bf16 -> [128, n_c1, 8, FD1] with xT[p, c, d, t] =
    xs[c*FD1 + t, d*128 + p]."""
    cap = xs.shape[0]
    a = xs.reshape(cap // FD1, FD1, n_d_host, P)  # (c, t, d, p)
    return np.ascontiguousarray(a.transpose(3, 0, 2, 1))


def _swizzle_wu(wu8):
    """[H, D] bf16 -> [128, 16, 8, 128] with wuT[p, j, d, t] =
    wu8[j*128 + t, d*128 + p]."""
    a = wu8.reshape(H // P, P, n_d_host, P)  # (j, t, d, p)
    return np.ascontiguousarray(a.transpose(3, 0, 2, 1))


def _swizzle_wd(wd8):
    """[D, H] bf16 -> [128, 16, 1024] with wdT[p, j, i] =
    wd8[i, j*128 + p]."""
    a = wd8.T.reshape(H // P, P, D)  # (j, p, i)
    return np.ascontiguousarray(a.transpose(1, 0, 2))


def kernel(x, num_tokens_per_expert, w_up, w_down, _trace=False):
    global LAST_RESULT
    bf = ml_dtypes.bfloat16
    x = np.asarray(x)
    counts = np.asarray(num_tokens_per_expert).astype(np.int64)
    w_up = np.asarray(w_up)
    w_down = np.asarray(w_down)
    n_tok = x.shape[0]
    assert counts.shape == (E,) and int(counts.sum()) == n_tok
    offsets = np.zeros(E, dtype=np.int64)
    offsets[1:] = np.cumsum(counts)[:-1]

    nc = _get_program(CAP)

    # Work list: split each expert's contiguous token segment into slots of
    # <= CAP tokens; process 8 slots per SPMD round. The uniform T/E = 1024
    # split is exactly one round of 8 slots.
    slots = []
    for e in range(E):
        cnt, off = int(counts[e]), int(offsets[e])
        for s in range(0, cnt, CAP):
            slots.append((e, off + s, min(CAP, cnt - s)))

    wuT_cache = {}
    wdT_cache = {}

    def expert_weights(e):
        if e not in wuT_cache:
            wuT_cache[e] = _swizzle_wu(w_up[e].astype(bf))
            wdT_cache[e] = _swizzle_wd(w_down[e].astype(bf))
        return wuT_cache[e], wdT_cache[e]

    out = np.zeros((n_tok, D), dtype=x.dtype)
    zero_map = None
    for r0 in range(0, len(slots), N_CORES):
        round_slots = slots[r0:r0 + N_CORES]
        in_maps = []
        for e, off, cnt in round_slots:
            xs = np.zeros((CAP, D), dtype=bf)
            xs[:cnt] = x[off:off + cnt].astype(bf)
            wuT, wdT = expert_weights(e)
            in_maps.append({
                "xT": _swizzle_x(xs), "wuT": wuT, "wdT": wdT,
            })
        while len(in_maps) < N_CORES:  # idle cores in the last round
            if zero_map is None:
                zero_map = {
                    "xT": np.zeros((P, CAP // FD1, n_d_host, FD1), dtype=bf),
                    "wuT": np.zeros((P, H // P, n_d_host, P), dtype=bf),
                    "wdT": np.zeros((P, H // P, D), dtype=bf),
                }
            in_maps.append(zero_map)

        res = run_bass_kernel_spmd(
            nc, in_maps, core_ids=list(range(N_CORES)), trace=_trace
        )
        LAST_RESULT = res
        for i, (e, off, cnt) in enumerate(round_slots):
            out[off:off + cnt] = res.results[i]["y"][:cnt].astype(x.dtype)
    return out
